# revision 27
# baseline (speedup 1.0000x reference)
"""Multi-head attention (B=2, S=2048, D=1024, H=16) on 8 Trainium2 cores.

Sharding: core c handles batch b = c//4 and head group g = c%4 (4 heads each).
Each core computes its heads' attention output and a partial output
projection [S, D] in bf16; the host sums the 4 partials per batch (the
"all-reduce" after W_o done host-side).

All matmul operands are bf16 (fp32 runs 2-pass LOW_HIGH on the PE); PSUM
accumulation stays fp32.  The program is specialized on NKT =
max_b ceil(valid_len_b / 128): key tiles >= NKT are fully masked and
contribute exactly zero to both the softmax numerator and denominator
(the V/ones columns are pre-multiplied by the key mask z), so skipping
them is exact.  Compiled variants are cached per NKT.

Per-core pipeline (PSUM is the scarce resource - 8 banks):
  inputs: one 720 KB DMA per 128-row chunk of [X_b.T | Wqkv] (packed
    host-side), so projections start while later chunks stream in.
  phase 1a: Q.T = (s_b/8 * W_q) @ X.T and K.T = W_k @ X.T per head pair
    (4 PSUM banks); phase 1b: V token-major * z + z ones-column (2 banks).
  phase 2 runs per (q-half, head-pair) group, software-pipelined: stage A
  (scores+exp, 4 st banks) of group i overlaps stage B (PV+divide, 4 pp
  banks) of group i-1 on the PE.  Stage A of the first two groups is
  emitted inside the phase-1 pool windows (st 4 + 1a 4 banks, then
  st 4 + 1b 2), so the ScalarE exp stream starts as soon as the first
  head-pair's Q/K are ready instead of after all of phase 1.  Per kt:
    st_m [128, 1024] = scores.T, m = 512-wide q-span, head j in columns
      j*512:(j+1)*512 (its own bank); the two heads' score matmuls have
      64-row contractions at base partitions 0/64 emitted back-to-back,
      so the PE runs them concurrently via row tiling
    E.T = exp(st_m)  (ACTIVATE -> bf16 SBUF, parked for stage B)
    pp_j[0:64] += Vz_j.T @ E_j ; pp_j[64] += z.T @ E_j  (ones-column)
  divide: den row -> DMA-reshape [128,8] -> DVE recip (128 lanes) -> DMA
    back -> gpsimd partition_broadcast -> attn.T = pp[0:64]*recip (bf16)
  phase 4 (partial = attn @ W_o[:, rows].T) reuses the st PSUM tags, so
  its first half overlaps the trailing stage-B groups.
Edge case valid_len == 0: host sets s_b = 0, z = ones, NKT = 16 -> E = 1
  -> uniform attention over all keys, exactly matching the reference.
"""

import sys

if "/opt/trn_rl_repo" not in sys.path:
    sys.path.insert(0, "/opt/trn_rl_repo")

import numpy as np
from contextlib import ExitStack

import concourse.bass as bass
import concourse.tile as tile
from concourse import bacc, mybir
from concourse import bass_utils

F32 = mybir.dt.float32
BF16 = mybir.dt.bfloat16
EXP = mybir.ActivationFunctionType.Exp

B, S, D = 2, 2048, 1024
H, DK = 16, 64
HPC = 4            # heads per core
HC = HPC * DK      # head-group width = 256
N_CORES = 8
PT = 128           # partitions
NTT = S // PT      # 16 token tiles
NFC = D // PT      # 8 feature chunks
NQC = S // 512     # 4 q-chunks of 512
QH = 1024          # phase-2 q-half width


def _emit(tc, xt, wo, zt, out, nkt):
    nc = tc.nc
    SK = nkt * PT                       # active key span
    kws = [min(512, SK - c * 512) for c in range((SK + 511) // 512)]
    with ExitStack() as ctx:
        sb = ctx.enter_context(tc.tile_pool(name="sb", bufs=1))

        # ---- resident inputs ----
        wts, xts = [], []
        for fc in range(NFC):
            xw = sb.tile([PT, S + 3 * HC], BF16, name=f"xw{fc}")
            nc.sync.dma_start(xw[:], xt[fc * PT:(fc + 1) * PT, :])
            xts.append(xw[:, 0:S])
            wts.append(xw[:, S:S + 3 * HC])
        wos = []
        for c in range(2):
            t = sb.tile([PT, D], BF16, name=f"wos{c}")
            nc.sync.dma_start(t[:], wo[c * PT:(c + 1) * PT, :])
            wos.append(t)
        ztt = sb.tile([PT, NTT], F32, name="ztt")
        nc.sync.dma_start(ztt[:], zt[:])

        # ---- resident intermediates ----
        qsb = [sb.tile([PT, S], BF16, name=f"qsb{p}") for p in range(2)]
        ksb = [sb.tile([PT, SK], BF16, name=f"ksb{p}") for p in range(2)]
        vzs = [sb.tile([PT, nkt, DK + 1], BF16, name=f"vz{h}") for h in range(HPC)]
        attnT = [sb.tile([PT, S], BF16, name=f"attnT{c}") for c in range(2)]
        # warm the exp activation-table while the input DMA streams: write
        # into a corner of attnT that phase 2 later overwrites
        nc.scalar.activation(attnT[0][:, 0:NTT], ztt[:], EXP)

        with tc.tile_pool(name="ps_s", bufs=1, space="PSUM") as pss, \
             tc.tile_pool(name="etp", bufs=min(nkt + 4, 16)) as etp, \
             tc.tile_pool(name="upp", bufs=2) as upp, \
             tc.tile_pool(name="rpp", bufs=2) as rpp, \
             tc.tile_pool(name="bpp", bufs=2) as bpp, \
             tc.tile_pool(name="stg", bufs=3) as stg:

            def emit_1a(pq, p, qcs, with_k):
                # Q.T for the given 512-wide q-chunks (and K.T, first SK
                # cols, when with_k) for head pair p.  The q-halves needed
                # only by the late groups are emitted after 1b, off the
                # critical path to the early exp stream.
                groups = [(0, qsb[p], [(qc * 512, 512) for qc in qcs])]
                if with_k:
                    kcs = []
                    c0 = 0
                    for w in kws:
                        kcs.append((c0, w))
                        c0 += w
                    groups.append((HC, ksb[p], kcs))
                for off, dst, chunks in groups:
                    pts = [
                        pq.tile([PT, 512], F32, name="pqk", tag="pqk")
                        for _ in chunks
                    ]
                    for fc in range(NFC):
                        ws = wts[fc][:, off + p * PT:off + (p + 1) * PT]
                        for i, (c0, w) in enumerate(chunks):
                            nc.tensor.matmul(
                                pts[i][:, 0:w],
                                ws,
                                xts[fc][:, c0:c0 + w],
                                start=(fc == 0), stop=(fc == NFC - 1),
                            )
                    for i, (c0, w) in enumerate(chunks):
                        nc.vector.tensor_copy(dst[:, c0:c0 + w], pts[i][:, 0:w])

            def emit_1b_tt(pv, tt):
                # V token-major for one token tile, masked by z; shares the
                # pqk PSUM slots so it runs inside the 1a pool window
                pvt = pv.tile([PT, 512], F32, name="pvt", tag="pqk")[:, 0:HC]
                for fc in range(NFC):
                    nc.tensor.matmul(
                        pvt[:],
                        xts[fc][:, tt * PT:(tt + 1) * PT],
                        wts[fc][:, 2 * HC:3 * HC],
                        start=(fc == 0), stop=(fc == NFC - 1),
                    )
                for h in range(HPC):
                    nc.vector.tensor_scalar_mul(
                        vzs[h][:, tt, 0:DK],
                        pvt[:, h * DK:(h + 1) * DK],
                        ztt[:, tt:tt + 1],
                    )

            def emit_a_kt(g, kt):
                # stage A for one key tile: scores + exp; E parked in SBUF
                qh, p = g
                q0 = qh * QH
                pair = []
                for m in range(2):
                    stm = pss.tile([PT, QH], F32, name=f"st{m}", tag=f"st{m}")
                    for j in range(2):
                        nc.tensor.matmul(
                            stm[:, j * 512:(j + 1) * 512],
                            ksb[p][j * DK:(j + 1) * DK, kt * PT:(kt + 1) * PT],
                            qsb[p][j * DK:(j + 1) * DK, q0 + m * 512:q0 + (m + 1) * 512],
                            start=True, stop=True,
                        )
                    etm = etp.tile([PT, QH], BF16, name=f"et{m}", tag=f"et{m}")
                    nc.scalar.activation(etm[:], stm[:], EXP)
                    pair.append(etm)
                return pair

            def emit_a(g):
                return [emit_a_kt(g, kt) for kt in range(nkt)]

            def emit_b(psp, g, ets):
                # stage B: PV accumulation + normalization
                qh, p = g
                q0 = qh * QH
                pps = [
                    psp.tile([DK + 1, QH], F32, name=f"pp{j}", tag=f"pp{j}")
                    for j in range(2)
                ]
                for kt in range(nkt):
                    for m in range(2):
                        for j in range(2):
                            nc.tensor.matmul(
                                pps[j][:, m * 512:(m + 1) * 512],
                                vzs[2 * p + j][:, kt, :],
                                ets[kt][m][:, j * 512:(j + 1) * 512],
                                start=(kt == 0), stop=(kt == nkt - 1),
                            )
                for j in range(2):
                    po = j * DK
                    u = upp.tile([DK + 1, QH], F32, name=f"u{j}", tag=f"u{j}")
                    nc.vector.tensor_copy(u[:], pps[j][:])
                    # 1/den: DVE reciprocal is 8 cyc/elem per lane, so DMA
                    # the den row into [128, 8], recip on 128 lanes, DMA
                    # back, then broadcast across 64 partitions.
                    dv = rpp.tile([PT, QH // PT], F32, name="dv", tag=f"dv{j}")
                    nc.sync.dma_start(dv[:], u[DK:DK + 1, :])
                    nc.vector.reciprocal(dv[:], dv[:])
                    rr = rpp.tile([1, QH], F32, name="rr", tag=f"rr{j}")
                    nc.sync.dma_start(rr[:], dv[:])
                    rb = bpp.tile([DK, QH], F32, name="rb", tag=f"rb{j}")
                    nc.gpsimd.partition_broadcast(rb[:], rr[:])
                    nc.vector.tensor_mul(
                        attnT[p][po:po + DK, q0:q0 + QH], u[0:DK, :], rb[:]
                    )

            def emit_out(tts):
                # phase 4: partial = attn @ W_o[:, rows].T for token blocks
                # whose attnT columns are complete; pot reuses the st tags
                for tt in tts:
                    pot = pss.tile([PT, D], F32, name="pot", tag=f"st{tt % 2}")
                    for c in range(2):
                        for half in range(2):
                            nc.tensor.matmul(
                                pot[:, half * 512:(half + 1) * 512],
                                attnT[c][:, tt * PT:(tt + 1) * PT],
                                wos[c][:, half * 512:(half + 1) * 512],
                                start=(c == 0), stop=(c == 1),
                            )
                    so = stg.tile([PT, D], BF16, name="so", tag="so")
                    if tt % 2 == 0:
                        nc.vector.tensor_copy(so[:], pot[:])
                    else:
                        nc.scalar.copy(so[:], pot[:])
                    nc.sync.dma_start(out[tt * PT:(tt + 1) * PT, :], so[:])

            # PSUM co-residency: st(4) + 1a(4), then st(4) + 1b(2),
            # then st(4) + pp(4) - always <= 8 banks.
            with tc.tile_pool(name="ps_qk", bufs=4, space="PSUM") as pq:
                emit_1a(pq, 0, [0, 1], True)
                e00 = emit_a((0, 0))
                emit_1a(pq, 1, [0, 1], True)
                # V projection drains through A00's ACT-bound window so
                # B00's PV inputs are ready the moment A01 starts
                for tt in range(nkt):
                    emit_1b_tt(pq, tt)
                for h in range(HPC):
                    nc.vector.tensor_copy(vzs[h][:, :, DK], ztt[:, 0:nkt])
                # q-halves for the late groups (A10/A11), filler priority
                emit_1a(pq, 0, [2, 3], False)
                emit_1a(pq, 1, [2, 3], False)
            with tc.tile_pool(name="ps_p", bufs=1, space="PSUM") as psp:
                e01 = emit_a((0, 1))
                emit_b(psp, (0, 0), e00)
                e10 = emit_a((1, 0))
                emit_b(psp, (0, 1), e01)
                e11 = emit_a((1, 1))
                emit_out(range(0, NTT // 2))
                emit_b(psp, (1, 0), e10)
                emit_b(psp, (1, 1), e11)
                emit_out(range(NTT // 2, NTT))


def build(nkt=NTT):
    nc = bacc.Bacc(
        "TRN2",
        target_bir_lowering=False,
        debug=False,
        enable_asserts=True,
        num_devices=N_CORES,
    )
    xt = nc.dram_tensor("xt", [D, S + 3 * HC], BF16, kind="ExternalInput").ap()
    wo = nc.dram_tensor("wo", [HC, D], BF16, kind="ExternalInput").ap()
    zt = nc.dram_tensor("zt", [PT, NTT], F32, kind="ExternalInput").ap()
    out = nc.dram_tensor("out", [S, D], BF16, kind="ExternalOutput").ap()
    with tile.TileContext(nc) as tc:
        _emit(tc, xt, wo, zt, out, nkt)
    nc.compile()
    return nc


_NCS = {}


def _get_nc(nkt):
    if nkt not in _NCS:
        _NCS[nkt] = build(nkt)
    return _NCS[nkt]


def _nkt_for(vls):
    nkts = []
    for v in vls:
        v = int(v)
        nkts.append(NTT if v <= 0 else min(NTT, (v + PT - 1) // PT))
    return max(nkts)


def make_in_maps(X, valid_lens, W_q, W_k, W_v, W_o):
    import ml_dtypes

    bf16 = ml_dtypes.bfloat16
    X = np.asarray(X, dtype=np.float32)
    W_q = np.asarray(W_q, dtype=np.float32)
    W_k = np.asarray(W_k, dtype=np.float32)
    W_v = np.asarray(W_v, dtype=np.float32)
    W_o = np.asarray(W_o, dtype=np.float32)
    vls = np.asarray(valid_lens).astype(np.int64)
    in_maps = []
    for c in range(N_CORES):
        b, g = divmod(c, 4)
        rows = slice(g * HC, (g + 1) * HC)
        vl = int(vls[b])
        s = 0.125 if vl > 0 else 0.0
        if vl > 0:
            z = (np.arange(S) < vl).astype(np.float32)
        else:
            z = np.ones(S, dtype=np.float32)
        wqkv = np.concatenate(
            [W_q[rows].T * s, W_k[rows].T, W_v[rows].T], axis=1
        )
        xtw = np.concatenate([X[b].T, wqkv], axis=1)
        in_maps.append({
            "xt": np.ascontiguousarray(xtw).astype(bf16),
            "wo": np.ascontiguousarray(W_o.T[rows]).astype(bf16),
            "zt": np.ascontiguousarray(z.reshape(NTT, PT).T),
        })
    return in_maps


def combine(outs):
    out = np.empty((B, S, D), dtype=np.float32)
    for b in range(B):
        out[b] = (
            np.asarray(outs[4 * b], np.float32)
            + np.asarray(outs[4 * b + 1], np.float32)
            + np.asarray(outs[4 * b + 2], np.float32)
            + np.asarray(outs[4 * b + 3], np.float32)
        )
    return out


def kernel(X, valid_lens, W_q, W_k, W_v, W_o):
    vls = np.asarray(valid_lens).astype(np.int64)
    nc = _get_nc(_nkt_for(vls))
    in_maps = make_in_maps(X, valid_lens, W_q, W_k, W_v, W_o)
    res = bass_utils.run_bass_kernel_spmd(nc, in_maps, core_ids=list(range(N_CORES)))
    return combine([r["out"] for r in res.results])
